# revision 15
# baseline (speedup 1.0000x reference)
"""Trainium2 Bass kernel for multi-head attention (B=2, S=2048, D=1024, H=16, causal, RoPE).

Sharding: tensor-parallel over heads. Each of the 8 cores computes 2 heads
(128 of the 1024 q/k/v dims): QKV projections for its head slice, RoPE,
causal attention, and a partial output projection against its 128-column
slice of o_weight. The host sums the 8 partial outputs (the all-reduce).

v2 design notes (cost-model driven):
  - QKV projections run as fp8e4 DoubleRow matmuls (2 fp8 rows/cycle) with
    full error compensation: host splits each weight slice into
    W_hi + W_lo (both fp8, prescaled by SC=32) and streams x as
    x8 + x8lo (fp8 + scaled fp8 residual). q = x8@(Wh+Wl) + x8lo@Wh16,
    three DoubleRow chains accumulating in one PSUM tile -> 0.75x the
    bf16 matmul cost at bf16-level accuracy.
  - q and k project into one [128,2,512] PSUM tile; RoPE computes
    sin-product first (msin = psum * sin), rotates it through the sperm
    matmul (sign folded into sperm), then adds the cos-product - this
    avoids a separate PSUM->SBUF copy for the pre-rotation values.
  - Rotated q/k are written as fp8 and DMA-reshuffled (SBUF->SBUF) into
    the [32, 2-ktile, {q,k}, seq] layout that DoubleRow scores need:
    scores = K^T Q runs as one fp8 DR matmul per (tile, head) at 0.5
    cycles/column - 2x the bf16 score rate.
  - V projects in the transposed [dh, seq] orientation (same cheap DR
    shape as q/k), then a DMA-transpose instruction moves it into the
    [seq, dh] layout PV needs. V block layout [vA |ones| vB |ones] makes
    both heads' PV outputs [num; den], so the softmax denominators land
    merged on partitions 64:128 and normalize is reciprocal+2 muls.
  - exp writes P in bf16 (PV runs bf16: fp8 P fails the error budget).
  - Work is software-pipelined as in v1: projections run one chunk ahead
    of attention, output projections trail one chunk behind.
"""

import numpy as np

D_MODEL = 1024
N_HEADS = 16
D_HEAD = 64
THETA = 10000.0
B = 2
S = 2048
N_CORES = 8
BS = B * S  # 4096
NQ = 512    # query chunk width
NK = 128    # key tile width
SC = 32.0   # weight prescale into a good e4m3 binade
RS = 8.0    # x-residual prescale

_RT = {}


def _build():
    if _RT:
        return _RT
    import sys
    try:
        import concourse.bass  # noqa: F401
    except ImportError:
        sys.path.insert(0, "/opt/trn_rl_repo")
    import concourse.mybir as mybir
    import concourse.tile as tile
    from concourse import bacc
    from concourse._compat import axon_active
    from concourse.bass_utils import run_bass_kernel_spmd

    f32 = mybir.dt.float32
    f32r = mybir.dt.float32r
    bf16 = mybir.dt.bfloat16
    f8 = mybir.dt.float8e4
    EXP = mybir.ActivationFunctionType.Exp
    DR = mybir.MatmulPerfMode.DoubleRow

    nc = bacc.Bacc(
        "TRN2", target_bir_lowering=False, debug=not axon_active(),
        num_devices=N_CORES,
    )

    x8 = nc.dram_tensor("x8", [D_MODEL, BS], f8, kind="ExternalInput").ap()
    xlo = nc.dram_tensor("xlo", [D_MODEL, BS], f8, kind="ExternalInput").ap()
    WKEYS = [f"w{n}{p}" for n in ("q", "k", "v") for p in ("h", "l", "g")]
    w8 = {key: nc.dram_tensor(key, [D_MODEL, 128], f8, kind="ExternalInput").ap()
          for key in WKEYS}
    wo = nc.dram_tensor("wo", [128, D_MODEL], bf16, kind="ExternalInput").ap()
    trig = nc.dram_tensor("trig", [128, 2, S], f32, kind="ExternalInput").ap()
    sperm = nc.dram_tensor("sperm", [128, 128], f32r, kind="ExternalInput").ap()
    mask128 = nc.dram_tensor("mask128", [128, 128], bf16, kind="ExternalInput").ap()
    y = nc.dram_tensor("y", [BS, D_MODEL], bf16, kind="ExternalOutput").ap()

    with tile.TileContext(nc) as tc:
        with (
            tc.tile_pool(name="singles", bufs=1) as singles,
            tc.tile_pool(name="px", bufs=4) as px,
            tc.tile_pool(name="ptmp", bufs=3) as ptmp,
            tc.tile_pool(name="pp", bufs=4) as pp,
            tc.tile_pool(name="pys", bufs=4) as pys,
            tc.tile_pool(name="pr", bufs=2) as pr,
            tc.tile_pool(name="ps_a", bufs=2, space="PSUM") as ps_a,
            tc.tile_pool(name="ps_s", bufs=2, space="PSUM") as ps_s,
            tc.tile_pool(name="ps_o", bufs=1, space="PSUM") as ps_o,
        ):
            w_sb = {key: singles.tile([128, 4, 2, 128], f8, tag=key, name=key)
                    for key in WKEYS}
            wo_sb = singles.tile([128, D_MODEL], bf16, tag="wo")
            sperm_sb = singles.tile([128, 128], f32r, tag="sperm")
            m128_sb = singles.tile([128, 128], bf16, tag="m128")
            # q/k for scores, DoubleRow layout: partition = 32*head +
            # freq, dims = [ktile(2), {q,k}, batch*S + seq]
            qk8_sb = singles.tile([64, 2, 2, BS], f8, tag="qk8")
            # V tiles: [seq-tile partitions, 32 tiles, 256]:
            # [vA(0:64) | ones | vB(128:192) | ones]; head A lhsT = cols
            # 0:128, head B lhsT = cols 128:256 -> both PV outs [num; den].
            v_sb = singles.tile([128, 32, 256], bf16, tag="v")
            oT_sb = singles.tile([128, BS], bf16, tag="oT")

            for key in WKEYS:
                nc.scalar.dma_start(
                    out=w_sb[key],
                    in_=w8[key].rearrange("(j t p) m -> p j t m", j=4, t=2))
            nc.scalar.dma_start(out=sperm_sb, in_=sperm)
            nc.scalar.dma_start(out=m128_sb, in_=mask128)
            nc.vector.memset(v_sb[:, :, 64:128], 1.0)
            nc.vector.memset(v_sb[:, :, 192:256], 1.0)

            def late_consts():
                nc.scalar.dma_start(out=wo_sb, in_=wo)

            def proj_pieces(b, c, after_xt=None):
                """QKV projections + rope for seq chunk c of batch b (512
                positions), as a list of closures threaded through the
                attention tile loop."""
                u = 4 * b + c
                s0 = NQ * c
                csl = slice(NQ * u, NQ * (u + 1))
                st = {}

                def p_load():
                    st["tg"] = ptmp.tile([128, 2, NQ], f32, tag="tg", name="tg")
                    nc.sync.dma_start(out=st["tg"], in_=trig[:, :, s0:s0 + NQ])
                    st["x8"] = px.tile([128, 8, NQ], f8, tag="x8", name="x8t")
                    nc.sync.dma_start(
                        out=st["x8"],
                        in_=x8[:, csl].rearrange("(a p) n -> p a n", p=128))
                    st["xlo"] = px.tile([128, 8, NQ], f8, tag="xlo", name="xlot")
                    nc.sync.dma_start(
                        out=st["xlo"],
                        in_=xlo[:, csl].rearrange("(a p) n -> p a n", p=128))
                    if after_xt is not None:
                        after_xt()

                def p_proj(which, h):
                    def f():
                        ps = ps_a.tile([128, NQ], f32, tag="pa",
                                       name=f"{which}ps")
                        st[which] = ps
                        n = 0
                        for wk, xk in ((f"w{which}h", "x8"),
                                       (f"w{which}l", "x8"),
                                       (f"w{which}g", "xlo")):
                            for j in range(4):
                                nc.tensor.matmul(
                                    ps, w_sb[wk][:, j],
                                    st[xk][:, 2 * j:2 * j + 2, :],
                                    start=(n == 0), stop=(n == 11),
                                    perf_mode=DR)
                                n += 1
                    return f

                def p_rope_sin(which, h):
                    def f():
                        st[f"msin{h}"] = ptmp.tile([128, NQ], f32r, tag="msin",
                                                   name="msin")
                        nc.vector.tensor_mul(st[f"msin{h}"], st[which],
                                             st["tg"][:, 1, :])
                    return f

                def p_rope_cos(which, h):
                    def f():
                        if "qkstr" not in st:
                            st["qkstr"] = ptmp.tile([128, 2, NQ], f8,
                                                    tag="qkstr", name="qkstr")
                        st[f"m1{h}"] = ptmp.tile([128, NQ], f32, tag="m1",
                                                 name="m1")
                        nc.vector.tensor_mul(st[f"m1{h}"], st[which],
                                             st["tg"][:, 0, :])
                    return f

                def p_rope_mm(h):
                    def f():
                        st[f"sq{h}"] = ps_a.tile([128, NQ], f32, tag="pa",
                                                 name="sqps")
                        nc.tensor.matmul(st[f"sq{h}"], sperm_sb,
                                         st[f"msin{h}"], start=True, stop=True)
                    return f

                def p_rope_add(h):
                    def f():
                        nc.vector.tensor_add(st["qkstr"][:, h, :],
                                             st[f"m1{h}"], st[f"sq{h}"])
                    return f

                def p_qk_dma(k):
                    def f():
                        g0 = S * b + s0
                        for h in range(2):
                            for t in range(2):
                                p0 = 64 * h + 32 * t
                                nc.sync.dma_start(
                                    out=qk8_sb[32 * h:32 * h + 32, t, k,
                                               g0:g0 + NQ],
                                    in_=st["qkstr"][p0:p0 + 32, k, :])
                    return f

                def p_projv():
                    st["v"] = ps_a.tile([128, NQ], f32, tag="pa", name="vps")
                    n = 0
                    for wk, xk in (("wvh", "x8"), ("wvl", "x8"), ("wvg", "xlo")):
                        for j in range(4):
                            nc.tensor.matmul(
                                st["v"], w_sb[wk][:, j],
                                st[xk][:, 2 * j:2 * j + 2, :],
                                start=(n == 0), stop=(n == 11), perf_mode=DR)
                            n += 1

                def p_vcopy():
                    st["vt"] = ptmp.tile([128, NQ], bf16, tag="vt", name="vt")
                    nc.scalar.copy(st["vt"], st["v"])

                def p_vdma_a():
                    nc.sync.dma_start(out=v_sb[:, 4 * u:4 * u + 4, 0:64],
                                      in_=st["vt"][0:64, :], transpose=True)

                def p_vdma_b():
                    nc.sync.dma_start(out=v_sb[:, 4 * u:4 * u + 4, 128:192],
                                      in_=st["vt"][64:128, :], transpose=True)

                return [p_load,
                        p_proj("q", 0), p_rope_sin("q", 0), p_rope_cos("q", 0),
                        p_rope_mm(0), p_rope_add(0), p_qk_dma(0),
                        p_proj("k", 1), p_rope_sin("k", 1), p_rope_cos("k", 1),
                        p_rope_mm(1), p_rope_add(1), p_qk_dma(1),
                        p_projv, p_vcopy, p_vdma_a, p_vdma_b]

            def proj_chunk(b, c, after_xt=None):
                for f in proj_pieces(b, c, after_xt):
                    f()

            def oproj_piece(b, c, s4):
                """Output projection for one 128-row seq tile (emitted one
                chunk late, spread across the next chunk's tiles)."""
                row0 = S * b + NQ * c + 128 * s4
                yp = ps_s.tile([128, 2, NQ], f32, tag="sps")
                for hn in range(2):
                    nc.tensor.matmul(
                        yp[:, hn, :],
                        oT_sb[:, row0:row0 + 128],
                        wo_sb[:, NQ * hn:NQ * (hn + 1)],
                        start=True, stop=True,
                    )
                ys = pys.tile([128, D_MODEL], bf16, tag="ys")
                if s4 % 2 == 0:
                    nc.vector.tensor_copy(ys, yp.rearrange("p a n -> p (a n)"))
                else:
                    nc.scalar.copy(ys, yp.rearrange("p a n -> p (a n)"))
                nc.sync.dma_start(out=y[row0:row0 + 128, :], in_=ys)

            def attn_chunk(b, c, mids=()):
                """Causal attention for query chunk c of batch b. ``mids`` are
                emitted one per attention tile (pipelined filler work)."""
                mids = list(mids)
                q0 = NQ * c
                qsl = slice(S * b + q0, S * b + q0 + NQ)
                nt = 4 * (c + 1)
                oab = ps_o.tile([128, 2, NQ], f32, tag="oacc")
                pending = []  # (p tile, j, t) awaiting PV matmul
                PV_DEPTH = 2

                def pv_flush():
                    p, j, t = pending.pop(0)
                    w0 = 128 * j
                    vt = v_sb[:, 16 * b + t, :]
                    nc.tensor.matmul(
                        oab[:, 0, w0:NQ], vt[:, 0:128], p[:, 0, w0:NQ],
                        start=(t == 0), stop=(t == nt - 1),
                    )
                    nc.tensor.matmul(
                        oab[:, 1, w0:NQ], vt[:, 128:256], p[:, 1, w0:NQ],
                        start=(t == 0), stop=(t == nt - 1),
                    )

                for t in range(nt):
                    j = max(0, t - 4 * c)  # within-chunk diagonal offset
                    w0 = 128 * j           # causally-dead query columns
                    sps = ps_s.tile([128, 2, NQ], f32, tag="sps")
                    for h in range(2):
                        base = 32 * h
                        g0 = S * b
                        nc.tensor.matmul(
                            sps[:, h, w0:NQ],
                            qk8_sb[base:base + 32, :, 1,
                                   g0 + NK * t:g0 + NK * (t + 1)],
                            qk8_sb[base:base + 32, :, 0,
                                   g0 + q0 + w0:g0 + q0 + NQ],
                            start=True, stop=True, perf_mode=DR)
                    p = pp.tile([128, 2, NQ], bf16, tag="p")
                    nc.scalar.activation(
                        p[:, :, w0:NQ], sps[:, :, w0:NQ], EXP, scale=0.125,
                    )
                    if t >= 4 * c:  # diagonal tile: mask boundary block
                        pb = p[:, :, w0:w0 + 128]
                        nc.vector.tensor_mul(
                            pb, pb,
                            m128_sb.unsqueeze(1).to_broadcast([128, 2, 128]),
                        )
                    if len(pending) >= PV_DEPTH:
                        pv_flush()
                    pending.append((p, j, t))
                    if mids:
                        mids.pop(0)()
                while pending:
                    pv_flush()
                for m in mids:  # in case nt < len(mids)
                    m()

                # both heads' denominators sit replicated on partitions
                # 64:128 of oab (ones blocks in V); normalize per half
                # chunk so the trailing output projection can start early.
                rr = pr.tile([64, 2, NQ], f32, tag="rr")
                nc.vector.reciprocal(rr, oab[64:128, :, :])
                nc.vector.tensor_mul(oT_sb[0:64, qsl], oab[0:64, 0, :],
                                     rr[:, 0, :])
                nc.vector.tensor_mul(oT_sb[64:128, qsl], oab[0:64, 1, :],
                                     rr[:, 1, :])

            # Software pipeline: projections run one chunk ahead of attention;
            # output projections trail their attention chunk by one.
            def oproj_mids(bc):
                if bc is None:
                    return ()
                return [lambda s4=s4: oproj_piece(bc[0], bc[1], s4)
                        for s4 in range(4)]

            prev = None  # (b, c) whose oproj is still owed
            for b in range(B):
                if b == 0:
                    proj_chunk(b, 0, after_xt=late_consts)
                for c in range(4):
                    mids = list(oproj_mids(prev))
                    if c + 1 < 4:
                        pieces = proj_pieces(b, c + 1)
                    elif b + 1 < B:
                        pieces = proj_pieces(b + 1, 0)
                    else:
                        pieces = []
                    merged = []
                    while pieces or mids:
                        if pieces:
                            merged.append(pieces.pop(0))
                        if mids:
                            merged.append(mids.pop(0))
                    attn_chunk(b, c, mids=merged)
                    prev = (b, c)
            for s4 in range(4):
                oproj_piece(prev[0], prev[1], s4)

    nc.compile()
    _RT.update(
        nc=nc, run_bass_kernel_spmd=run_bass_kernel_spmd, mybir=mybir,
    )
    return _RT


def _host_inputs(q_weight, k_weight, v_weight, o_weight, in_features):
    """Build the per-core input maps (host-side sharding + layout prep)."""
    import ml_dtypes
    f8 = ml_dtypes.float8_e4m3fn
    bf = ml_dtypes.bfloat16

    x = np.asarray(in_features, dtype=np.float32).reshape(BS, D_MODEL)
    xT = np.ascontiguousarray(x.T)
    x8 = xT.astype(f8)
    xlo = ((xT - x8.astype(np.float32)) * RS).astype(f8)

    qw = np.asarray(q_weight, dtype=np.float32)
    kw = np.asarray(k_weight, dtype=np.float32)
    vw = np.asarray(v_weight, dtype=np.float32)
    ow = np.asarray(o_weight, dtype=np.float32)

    perm64 = np.concatenate([np.arange(0, 64, 2), np.arange(1, 64, 2)])

    half = D_HEAD // 2
    inv_freq = THETA ** (-(np.arange(half, dtype=np.float64) * 2.0 / D_HEAD))
    pos = np.arange(S, dtype=np.float64)
    ang = pos[None, :] * inv_freq[:, None]        # [32, S]
    angf = np.tile(ang, (4, 1))                   # [128, S], row p -> i = p % 32
    trig = np.ascontiguousarray(
        (np.stack([np.cos(angf), np.sin(angf)], axis=1) / SC).astype(np.float32))

    spermT = np.zeros((128, 128), dtype=np.float32)
    for h in range(2):
        for i in range(32):
            spermT[h * 64 + 32 + i, h * 64 + i] = -1.0
            spermT[h * 64 + i, h * 64 + 32 + i] = 1.0

    kq = np.arange(128)
    mask128 = (np.arange(128)[None, :] >= kq[:, None]).astype(bf)

    shared = dict(x8=x8, xlo=xlo, trig=trig, sperm=spermT, mask128=mask128)

    def wsplit(A):
        # A: [1024, 128] f32, prescaled by SC
        wh = A.astype(f8)
        wl = (A - wh.astype(np.float32)).astype(f8)
        wg = (A / RS).astype(f8)
        return (np.ascontiguousarray(wh), np.ascontiguousarray(wl),
                np.ascontiguousarray(wg))

    in_maps = []
    for cidx in range(N_CORES):
        rows = slice(128 * cidx, 128 * (cidx + 1))

        def permqk(w):
            wc = w[rows]
            return np.concatenate([wc[0:64][perm64], wc[64:128][perm64]]).T * SC

        m = dict(shared)
        for nm, w in (("q", qw), ("k", kw)):
            A = permqk(w)
            for suff, arr in zip("hlg", wsplit(A)):
                m[f"w{nm}{suff}"] = arr
        Av = vw[rows].T * SC
        for suff, arr in zip("hlg", wsplit(Av)):
            m[f"wv{suff}"] = arr
        m["wo"] = np.ascontiguousarray(ow[:, rows].T / SC).astype(bf)
        in_maps.append(m)
    return in_maps


def kernel(q_weight, k_weight, v_weight, o_weight, in_features):
    rt = _build()
    in_maps = _host_inputs(q_weight, k_weight, v_weight, o_weight, in_features)
    res = rt["run_bass_kernel_spmd"](
        rt["nc"], in_maps, core_ids=list(range(N_CORES)),
    )
    y = np.zeros((BS, D_MODEL), dtype=np.float32)
    for c in range(N_CORES):
        y += np.asarray(res.results[c]["y"], dtype=np.float32)
    return y.reshape(B, S, D_MODEL)


# revision 16
# speedup vs baseline: 1.0633x; 1.0633x over previous
"""Trainium2 Bass kernel for multi-head attention (B=2, S=2048, D=1024, H=16, causal, RoPE).

Sharding: tensor-parallel over heads. Each of the 8 cores computes 2 heads
(128 of the 1024 q/k/v dims): QKV projections for its head slice, RoPE,
causal attention, and a partial output projection against its 128-column
slice of o_weight. The host sums the 8 partial outputs (the all-reduce).

v2 design notes (cost-model driven):
  - QKV projections run as fp8e4 DoubleRow matmuls (2 fp8 rows/cycle) with
    full error compensation: host splits each weight slice into
    W_hi + W_lo (both fp8, prescaled by SC=32) and streams x as
    x8 + x8lo (fp8 + scaled fp8 residual). q = x8@(Wh+Wl) + x8lo@Wh16,
    three DoubleRow chains accumulating in one PSUM tile -> 0.75x the
    bf16 matmul cost at bf16-level accuracy.
  - q and k project into one [128,2,512] PSUM tile; RoPE computes
    sin-product first (msin = psum * sin), rotates it through the sperm
    matmul (sign folded into sperm), then adds the cos-product - this
    avoids a separate PSUM->SBUF copy for the pre-rotation values.
  - Rotated q/k are written as fp8 and DMA-reshuffled (SBUF->SBUF) into
    the [32, 2-ktile, {q,k}, seq] layout that DoubleRow scores need:
    scores = K^T Q runs as one fp8 DR matmul per (tile, head) at 0.5
    cycles/column - 2x the bf16 score rate.
  - V projects in the transposed [dh, seq] orientation (same cheap DR
    shape as q/k), then a DMA-transpose instruction moves it into the
    [seq, dh] layout PV needs. V block layout [vA |ones| vB |ones] makes
    both heads' PV outputs [num; den], so the softmax denominators land
    merged on partitions 64:128 and normalize is reciprocal+2 muls.
  - exp writes P in bf16 (PV runs bf16: fp8 P fails the error budget).
  - Work is software-pipelined as in v1: projections run one chunk ahead
    of attention, output projections trail one chunk behind.
"""

import numpy as np

D_MODEL = 1024
N_HEADS = 16
D_HEAD = 64
THETA = 10000.0
B = 2
S = 2048
N_CORES = 8
BS = B * S  # 4096
NQ = 512    # query chunk width
NK = 128    # key tile width
SC = 32.0   # weight prescale into a good e4m3 binade
RS = 8.0    # x-residual prescale

_RT = {}


def _build():
    if _RT:
        return _RT
    import sys
    try:
        import concourse.bass  # noqa: F401
    except ImportError:
        sys.path.insert(0, "/opt/trn_rl_repo")
    import concourse.mybir as mybir
    import concourse.tile as tile
    from concourse import bacc
    from concourse._compat import axon_active
    from concourse.bass_utils import run_bass_kernel_spmd

    f32 = mybir.dt.float32
    f32r = mybir.dt.float32r
    bf16 = mybir.dt.bfloat16
    f8 = mybir.dt.float8e4
    EXP = mybir.ActivationFunctionType.Exp
    DR = mybir.MatmulPerfMode.DoubleRow

    nc = bacc.Bacc(
        "TRN2", target_bir_lowering=False, debug=not axon_active(),
        num_devices=N_CORES,
    )

    x8 = nc.dram_tensor("x8", [D_MODEL, BS], f8, kind="ExternalInput").ap()
    xlo = nc.dram_tensor("xlo", [D_MODEL, BS], f8, kind="ExternalInput").ap()
    WKEYS = [f"w{n}{p}" for n in ("q", "k", "v") for p in ("h", "l", "g")]
    w8 = {key: nc.dram_tensor(key, [D_MODEL, 128], f8, kind="ExternalInput").ap()
          for key in WKEYS}
    wo = nc.dram_tensor("wo", [128, D_MODEL], bf16, kind="ExternalInput").ap()
    trig = nc.dram_tensor("trig", [128, 2, S], f32, kind="ExternalInput").ap()
    sperm = nc.dram_tensor("sperm", [128, 128], f32r, kind="ExternalInput").ap()
    mask128 = nc.dram_tensor("mask128", [128, 128], bf16, kind="ExternalInput").ap()
    y = nc.dram_tensor("y", [BS, D_MODEL], bf16, kind="ExternalOutput").ap()

    with tile.TileContext(nc) as tc:
        with (
            tc.tile_pool(name="singles", bufs=1) as singles,
            tc.tile_pool(name="px", bufs=4) as px,
            tc.tile_pool(name="ptmp", bufs=3) as ptmp,
            tc.tile_pool(name="pp", bufs=4) as pp,
            tc.tile_pool(name="pys", bufs=4) as pys,
            tc.tile_pool(name="pr", bufs=2) as pr,
            tc.tile_pool(name="ps_a", bufs=2, space="PSUM") as ps_a,
            tc.tile_pool(name="ps_s", bufs=2, space="PSUM") as ps_s,
            tc.tile_pool(name="ps_o", bufs=1, space="PSUM") as ps_o,
        ):
            w_sb = {key: singles.tile([128, 4, 2, 128], f8, tag=key, name=key)
                    for key in WKEYS}
            wo_sb = singles.tile([128, D_MODEL], bf16, tag="wo")
            sperm_sb = singles.tile([128, 128], f32r, tag="sperm")
            m128_sb = singles.tile([128, 128], bf16, tag="m128")
            # q/k for scores, DoubleRow layout: partition = 32*head +
            # freq, dims = [ktile(2), {q,k}, batch*S + seq]
            qk8_sb = singles.tile([64, 2, 2, BS], f8, tag="qk8")
            # V tiles: [seq-tile partitions, 32 tiles, 256]:
            # [vA(0:64) | ones | vB(128:192) | ones]; head A lhsT = cols
            # 0:128, head B lhsT = cols 128:256 -> both PV outs [num; den].
            v_sb = singles.tile([128, 32, 256], bf16, tag="v")
            oT_sb = singles.tile([128, BS], bf16, tag="oT")

            for key in WKEYS:
                nc.scalar.dma_start(
                    out=w_sb[key],
                    in_=w8[key].rearrange("(j t p) m -> p j t m", j=4, t=2))
            nc.scalar.dma_start(out=sperm_sb, in_=sperm)
            nc.scalar.dma_start(out=m128_sb, in_=mask128)
            nc.vector.memset(v_sb[:, :, 64:128], 1.0)
            nc.vector.memset(v_sb[:, :, 192:256], 1.0)

            def late_consts():
                nc.scalar.dma_start(out=wo_sb, in_=wo)

            def proj_pieces(b, c, after_xt=None):
                """QKV projections + rope for seq chunk c of batch b (512
                positions), as a list of closures threaded through the
                attention tile loop."""
                u = 4 * b + c
                s0 = NQ * c
                csl = slice(NQ * u, NQ * (u + 1))
                st = {}

                def p_load():
                    st["tg"] = ptmp.tile([128, 2, NQ], f32, tag="tg", name="tg")
                    nc.sync.dma_start(out=st["tg"], in_=trig[:, :, s0:s0 + NQ])
                    st["x8"] = px.tile([128, 8, NQ], f8, tag="x8", name="x8t")
                    nc.sync.dma_start(
                        out=st["x8"],
                        in_=x8[:, csl].rearrange("(a p) n -> p a n", p=128))
                    st["xlo"] = px.tile([128, 8, NQ], f8, tag="xlo", name="xlot")
                    nc.sync.dma_start(
                        out=st["xlo"],
                        in_=xlo[:, csl].rearrange("(a p) n -> p a n", p=128))
                    if after_xt is not None:
                        after_xt()

                def p_proj(which, h):
                    def f():
                        ps = ps_a.tile([128, NQ], f32, tag="pa",
                                       name=f"{which}ps")
                        st[which] = ps
                        n = 0
                        for wk, xk in ((f"w{which}h", "x8"),
                                       (f"w{which}l", "x8"),
                                       (f"w{which}g", "xlo")):
                            for j in range(4):
                                nc.tensor.matmul(
                                    ps, w_sb[wk][:, j],
                                    st[xk][:, 2 * j:2 * j + 2, :],
                                    start=(n == 0), stop=(n == 11),
                                    perf_mode=DR)
                                n += 1
                    return f

                def p_rope_sin(which, h):
                    def f():
                        st[f"msin{h}"] = ptmp.tile([128, NQ], f32r, tag="msin",
                                                   name="msin")
                        nc.vector.tensor_mul(st[f"msin{h}"], st[which],
                                             st["tg"][:, 1, :])
                    return f

                def p_rope_cos(which, h):
                    def f():
                        if "qkstr" not in st:
                            st["qkstr"] = ptmp.tile([128, 2, NQ], f8,
                                                    tag="qkstr", name="qkstr")
                        st[f"m1{h}"] = ptmp.tile([128, NQ], f32, tag="m1",
                                                 name="m1")
                        nc.vector.tensor_mul(st[f"m1{h}"], st[which],
                                             st["tg"][:, 0, :])
                    return f

                def p_rope_mm(h):
                    def f():
                        st[f"sq{h}"] = ps_a.tile([128, NQ], f32, tag="pa",
                                                 name="sqps")
                        nc.tensor.matmul(st[f"sq{h}"], sperm_sb,
                                         st[f"msin{h}"], start=True, stop=True)
                    return f

                def p_rope_add(h):
                    def f():
                        nc.vector.tensor_add(st["qkstr"][:, h, :],
                                             st[f"m1{h}"], st[f"sq{h}"])
                    return f

                def p_qk_dma(k):
                    def f():
                        g0 = S * b + s0
                        for h in range(2):
                            for t in range(2):
                                p0 = 64 * h + 32 * t
                                nc.sync.dma_start(
                                    out=qk8_sb[32 * h:32 * h + 32, t, k,
                                               g0:g0 + NQ],
                                    in_=st["qkstr"][p0:p0 + 32, k, :])
                    return f

                def p_projv():
                    st["v"] = ps_a.tile([128, NQ], f32, tag="pa", name="vps")
                    n = 0
                    for wk, xk in (("wvh", "x8"), ("wvl", "x8"), ("wvg", "xlo")):
                        for j in range(4):
                            nc.tensor.matmul(
                                st["v"], w_sb[wk][:, j],
                                st[xk][:, 2 * j:2 * j + 2, :],
                                start=(n == 0), stop=(n == 11), perf_mode=DR)
                            n += 1

                def p_vcopy():
                    st["vt"] = ptmp.tile([128, NQ], bf16, tag="vt", name="vt")
                    nc.scalar.copy(st["vt"], st["v"])

                def p_vdma_a():
                    nc.sync.dma_start(out=v_sb[:, 4 * u:4 * u + 4, 0:64],
                                      in_=st["vt"][0:64, :], transpose=True)

                def p_vdma_b():
                    nc.sync.dma_start(out=v_sb[:, 4 * u:4 * u + 4, 128:192],
                                      in_=st["vt"][64:128, :], transpose=True)

                return [p_load,
                        p_proj("q", 0), p_rope_sin("q", 0), p_rope_cos("q", 0),
                        p_rope_mm(0), p_rope_add(0), p_qk_dma(0),
                        p_proj("k", 1), p_rope_sin("k", 1), p_rope_cos("k", 1),
                        p_rope_mm(1), p_rope_add(1), p_qk_dma(1),
                        p_projv, p_vcopy, p_vdma_a, p_vdma_b]

            def proj_chunk(b, c, after_xt=None):
                for f in proj_pieces(b, c, after_xt):
                    f()

            def oproj_piece(b, c, s4):
                """Output projection for one 128-row seq tile (emitted one
                chunk late, spread across the next chunk's tiles)."""
                row0 = S * b + NQ * c + 128 * s4
                yp = ps_s.tile([128, 2, NQ], f32, tag="sps")
                for hn in range(2):
                    nc.tensor.matmul(
                        yp[:, hn, :],
                        oT_sb[:, row0:row0 + 128],
                        wo_sb[:, NQ * hn:NQ * (hn + 1)],
                        start=True, stop=True,
                    )
                ys = pys.tile([128, D_MODEL], bf16, tag="ys")
                if s4 % 2 == 0:
                    nc.vector.tensor_copy(ys, yp.rearrange("p a n -> p (a n)"))
                else:
                    nc.scalar.copy(ys, yp.rearrange("p a n -> p (a n)"))
                nc.sync.dma_start(out=y[row0:row0 + 128, :], in_=ys)

            def attn_chunk(b, c, mids=()):
                """Causal attention for query chunk c of batch b. ``mids`` are
                emitted one per attention tile (pipelined filler work)."""
                mids = list(mids)
                q0 = NQ * c
                qsl = slice(S * b + q0, S * b + q0 + NQ)
                nt = 4 * (c + 1)
                oab = ps_o.tile([128, 2, NQ], f32, tag="oacc")
                pending = []  # (p tile, j, t) awaiting PV matmul
                PV_DEPTH = 2

                def pv_flush():
                    p, j, t = pending.pop(0)
                    w0 = 128 * j
                    vt = v_sb[:, 16 * b + t, :]
                    nc.tensor.matmul(
                        oab[:, 0, w0:NQ], vt[:, 0:128], p[:, 0, w0:NQ],
                        start=(t == 0), stop=(t == nt - 1),
                    )
                    nc.tensor.matmul(
                        oab[:, 1, w0:NQ], vt[:, 128:256], p[:, 1, w0:NQ],
                        start=(t == 0), stop=(t == nt - 1),
                    )

                for t in range(nt):
                    j = max(0, t - 4 * c)  # within-chunk diagonal offset
                    w0 = 128 * j           # causally-dead query columns
                    sps = ps_s.tile([128, 2, NQ], f32, tag="sps")
                    for h in range(2):
                        base = 32 * h
                        g0 = S * b
                        nc.tensor.matmul(
                            sps[:, h, w0:NQ],
                            qk8_sb[base:base + 32, :, 1,
                                   g0 + NK * t:g0 + NK * (t + 1)],
                            qk8_sb[base:base + 32, :, 0,
                                   g0 + q0 + w0:g0 + q0 + NQ],
                            start=True, stop=True, perf_mode=DR)
                    p = pp.tile([128, 2, NQ], bf16, tag="p")
                    nc.scalar.activation(
                        p[:, :, w0:NQ], sps[:, :, w0:NQ], EXP, scale=0.125,
                    )
                    if t >= 4 * c:  # diagonal tile: mask boundary block
                        pb = p[:, :, w0:w0 + 128]
                        nc.gpsimd.tensor_mul(
                            pb, pb,
                            m128_sb.unsqueeze(1).to_broadcast([128, 2, 128]),
                        )
                    if len(pending) >= PV_DEPTH:
                        pv_flush()
                    pending.append((p, j, t))
                    if mids:
                        mids.pop(0)()
                while pending:
                    pv_flush()
                for m in mids:  # in case nt < len(mids)
                    m()

                # both heads' denominators sit replicated on partitions
                # 64:128 of oab (ones blocks in V); normalize per half
                # chunk so the trailing output projection can start early.
                rr = pr.tile([64, 2, NQ], f32, tag="rr")
                for h2 in range(2):
                    sl = slice(256 * h2, 256 * (h2 + 1))
                    qh = slice(qsl.start + 256 * h2, qsl.start + 256 * (h2 + 1))
                    nc.vector.reciprocal(rr[:, :, sl], oab[64:128, :, sl])
                    nc.vector.tensor_mul(oT_sb[0:64, qh], oab[0:64, 0, sl],
                                         rr[:, 0, sl])
                    nc.vector.tensor_mul(oT_sb[64:128, qh], oab[0:64, 1, sl],
                                         rr[:, 1, sl])

            # Software pipeline: projections run one chunk ahead of attention;
            # output projections trail their attention chunk by one.
            def oproj_mids(bc):
                if bc is None:
                    return ()
                return [lambda s4=s4: oproj_piece(bc[0], bc[1], s4)
                        for s4 in range(4)]

            prev = None  # (b, c) whose oproj is still owed
            for b in range(B):
                if b == 0:
                    proj_chunk(b, 0, after_xt=late_consts)
                for c in range(4):
                    mids = list(oproj_mids(prev))
                    if c + 1 < 4:
                        pieces = proj_pieces(b, c + 1)
                    elif b + 1 < B:
                        pieces = proj_pieces(b + 1, 0)
                    else:
                        pieces = []
                    merged = []
                    while pieces or mids:
                        if pieces:
                            merged.append(pieces.pop(0))
                        if mids:
                            merged.append(mids.pop(0))
                    attn_chunk(b, c, mids=merged)
                    prev = (b, c)
            for s4 in range(4):
                oproj_piece(prev[0], prev[1], s4)

    nc.compile()
    _RT.update(
        nc=nc, run_bass_kernel_spmd=run_bass_kernel_spmd, mybir=mybir,
    )
    return _RT


def _host_inputs(q_weight, k_weight, v_weight, o_weight, in_features):
    """Build the per-core input maps (host-side sharding + layout prep)."""
    import ml_dtypes
    f8 = ml_dtypes.float8_e4m3fn
    bf = ml_dtypes.bfloat16

    x = np.asarray(in_features, dtype=np.float32).reshape(BS, D_MODEL)
    xT = np.ascontiguousarray(x.T)
    x8 = xT.astype(f8)
    xlo = ((xT - x8.astype(np.float32)) * RS).astype(f8)

    qw = np.asarray(q_weight, dtype=np.float32)
    kw = np.asarray(k_weight, dtype=np.float32)
    vw = np.asarray(v_weight, dtype=np.float32)
    ow = np.asarray(o_weight, dtype=np.float32)

    perm64 = np.concatenate([np.arange(0, 64, 2), np.arange(1, 64, 2)])

    half = D_HEAD // 2
    inv_freq = THETA ** (-(np.arange(half, dtype=np.float64) * 2.0 / D_HEAD))
    pos = np.arange(S, dtype=np.float64)
    ang = pos[None, :] * inv_freq[:, None]        # [32, S]
    angf = np.tile(ang, (4, 1))                   # [128, S], row p -> i = p % 32
    trig = np.ascontiguousarray(
        (np.stack([np.cos(angf), np.sin(angf)], axis=1) / SC).astype(np.float32))

    spermT = np.zeros((128, 128), dtype=np.float32)
    for h in range(2):
        for i in range(32):
            spermT[h * 64 + 32 + i, h * 64 + i] = -1.0
            spermT[h * 64 + i, h * 64 + 32 + i] = 1.0

    kq = np.arange(128)
    mask128 = (np.arange(128)[None, :] >= kq[:, None]).astype(bf)

    shared = dict(x8=x8, xlo=xlo, trig=trig, sperm=spermT, mask128=mask128)

    def wsplit(A):
        # A: [1024, 128] f32, prescaled by SC
        wh = A.astype(f8)
        wl = (A - wh.astype(np.float32)).astype(f8)
        wg = (A / RS).astype(f8)
        return (np.ascontiguousarray(wh), np.ascontiguousarray(wl),
                np.ascontiguousarray(wg))

    in_maps = []
    for cidx in range(N_CORES):
        rows = slice(128 * cidx, 128 * (cidx + 1))

        def permqk(w):
            wc = w[rows]
            return np.concatenate([wc[0:64][perm64], wc[64:128][perm64]]).T * SC

        m = dict(shared)
        for nm, w in (("q", qw), ("k", kw)):
            A = permqk(w)
            for suff, arr in zip("hlg", wsplit(A)):
                m[f"w{nm}{suff}"] = arr
        Av = vw[rows].T * SC
        for suff, arr in zip("hlg", wsplit(Av)):
            m[f"wv{suff}"] = arr
        m["wo"] = np.ascontiguousarray(ow[:, rows].T / SC).astype(bf)
        in_maps.append(m)
    return in_maps


def kernel(q_weight, k_weight, v_weight, o_weight, in_features):
    rt = _build()
    in_maps = _host_inputs(q_weight, k_weight, v_weight, o_weight, in_features)
    res = rt["run_bass_kernel_spmd"](
        rt["nc"], in_maps, core_ids=list(range(N_CORES)),
    )
    y = np.zeros((BS, D_MODEL), dtype=np.float32)
    for c in range(N_CORES):
        y += np.asarray(res.results[c]["y"], dtype=np.float32)
    return y.reshape(B, S, D_MODEL)


# revision 17
# speedup vs baseline: 1.0659x; 1.0025x over previous
"""Trainium2 Bass kernel for multi-head attention (B=2, S=2048, D=1024, H=16, causal, RoPE).

Sharding: tensor-parallel over heads. Each of the 8 cores computes 2 heads
(128 of the 1024 q/k/v dims): QKV projections for its head slice, RoPE,
causal attention, and a partial output projection against its 128-column
slice of o_weight. The host sums the 8 partial outputs (the all-reduce).

v2 design notes (cost-model driven):
  - QKV projections run as fp8e4 DoubleRow matmuls (2 fp8 rows/cycle) with
    full error compensation: host splits each weight slice into
    W_hi + W_lo (both fp8, prescaled by SC=32) and streams x as
    x8 + x8lo (fp8 + scaled fp8 residual). q = x8@(Wh+Wl) + x8lo@Wh16,
    three DoubleRow chains accumulating in one PSUM tile -> 0.75x the
    bf16 matmul cost at bf16-level accuracy.
  - q and k project into one [128,2,512] PSUM tile; RoPE computes
    sin-product first (msin = psum * sin), rotates it through the sperm
    matmul (sign folded into sperm), then adds the cos-product - this
    avoids a separate PSUM->SBUF copy for the pre-rotation values.
  - Rotated q/k are written as fp8 and DMA-reshuffled (SBUF->SBUF) into
    the [32, 2-ktile, {q,k}, seq] layout that DoubleRow scores need:
    scores = K^T Q runs as one fp8 DR matmul per (tile, head) at 0.5
    cycles/column - 2x the bf16 score rate.
  - V projects in the transposed [dh, seq] orientation (same cheap DR
    shape as q/k), then a DMA-transpose instruction moves it into the
    [seq, dh] layout PV needs. V block layout [vA |ones| vB |ones] makes
    both heads' PV outputs [num; den], so the softmax denominators land
    merged on partitions 64:128 and normalize is reciprocal+2 muls.
  - exp writes P in bf16 (PV runs bf16: fp8 P fails the error budget).
  - Work is software-pipelined as in v1: projections run one chunk ahead
    of attention, output projections trail one chunk behind.
"""

import numpy as np

D_MODEL = 1024
N_HEADS = 16
D_HEAD = 64
THETA = 10000.0
B = 2
S = 2048
N_CORES = 8
BS = B * S  # 4096
NQ = 512    # query chunk width
NK = 128    # key tile width
SC = 32.0   # weight prescale into a good e4m3 binade
RS = 8.0    # x-residual prescale

_RT = {}


def _build():
    if _RT:
        return _RT
    import sys
    try:
        import concourse.bass  # noqa: F401
    except ImportError:
        sys.path.insert(0, "/opt/trn_rl_repo")
    import concourse.mybir as mybir
    import concourse.tile as tile
    from concourse import bacc
    from concourse._compat import axon_active
    from concourse.bass_utils import run_bass_kernel_spmd

    f32 = mybir.dt.float32
    f32r = mybir.dt.float32r
    bf16 = mybir.dt.bfloat16
    f8 = mybir.dt.float8e4
    EXP = mybir.ActivationFunctionType.Exp
    DR = mybir.MatmulPerfMode.DoubleRow

    nc = bacc.Bacc(
        "TRN2", target_bir_lowering=False, debug=not axon_active(),
        num_devices=N_CORES,
    )

    x8 = nc.dram_tensor("x8", [D_MODEL, BS], f8, kind="ExternalInput").ap()
    xlo = nc.dram_tensor("xlo", [D_MODEL, BS], f8, kind="ExternalInput").ap()
    WKEYS = [f"w{n}{p}" for n in ("q", "k", "v") for p in ("h", "l", "g")]
    w8 = {key: nc.dram_tensor(key, [D_MODEL, 128], f8, kind="ExternalInput").ap()
          for key in WKEYS}
    wo = nc.dram_tensor("wo", [128, D_MODEL], bf16, kind="ExternalInput").ap()
    trig = nc.dram_tensor("trig", [128, 2, S], f32, kind="ExternalInput").ap()
    sperm = nc.dram_tensor("sperm", [128, 128], f32r, kind="ExternalInput").ap()
    mask128 = nc.dram_tensor("mask128", [128, 128], bf16, kind="ExternalInput").ap()
    y = nc.dram_tensor("y", [BS, D_MODEL], bf16, kind="ExternalOutput").ap()

    with tile.TileContext(nc) as tc:
        with (
            tc.tile_pool(name="singles", bufs=1) as singles,
            tc.tile_pool(name="px", bufs=4) as px,
            tc.tile_pool(name="ptmp", bufs=3) as ptmp,
            tc.tile_pool(name="pp", bufs=4) as pp,
            tc.tile_pool(name="pys", bufs=4) as pys,
            tc.tile_pool(name="pr", bufs=2) as pr,
            tc.tile_pool(name="ps_a", bufs=1, space="PSUM") as ps_a,
            tc.tile_pool(name="ps_s", bufs=2, space="PSUM") as ps_s,
            tc.tile_pool(name="ps_o", bufs=1, space="PSUM") as ps_o,
        ):
            w_sb = {key: singles.tile([128, 4, 2, 128], f8, tag=key, name=key)
                    for key in WKEYS}
            wo_sb = singles.tile([128, D_MODEL], bf16, tag="wo")
            sperm_sb = singles.tile([128, 128], f32r, tag="sperm")
            m128_sb = singles.tile([128, 128], bf16, tag="m128")
            # q/k for scores, DoubleRow layout: partition = 32*head +
            # freq, dims = [ktile(2), {q,k}, batch*S + seq]
            qk8_sb = singles.tile([64, 2, 2, BS], f8, tag="qk8")
            # V tiles: [seq-tile partitions, 32 tiles, 256]:
            # [vA(0:64) | ones | vB(128:192) | ones]; head A lhsT = cols
            # 0:128, head B lhsT = cols 128:256 -> both PV outs [num; den].
            v_sb = singles.tile([128, 32, 256], bf16, tag="v")
            oT_sb = singles.tile([128, BS], bf16, tag="oT")

            for key in WKEYS:
                nc.scalar.dma_start(
                    out=w_sb[key],
                    in_=w8[key].rearrange("(j t p) m -> p j t m", j=4, t=2))
            nc.scalar.dma_start(out=sperm_sb, in_=sperm)
            nc.scalar.dma_start(out=m128_sb, in_=mask128)
            nc.vector.memset(v_sb[:, :, 64:128], 1.0)
            nc.vector.memset(v_sb[:, :, 192:256], 1.0)

            def late_consts():
                nc.scalar.dma_start(out=wo_sb, in_=wo)

            def proj_pieces(b, c, after_xt=None):
                """QKV projections + rope for seq chunk c of batch b (512
                positions), as a list of closures threaded through the
                attention tile loop."""
                u = 4 * b + c
                s0 = NQ * c
                csl = slice(NQ * u, NQ * (u + 1))
                st = {}

                def p_load():
                    st["tg"] = ptmp.tile([128, 2, NQ], f32, tag="tg", name="tg")
                    nc.sync.dma_start(out=st["tg"], in_=trig[:, :, s0:s0 + NQ])
                    st["x8"] = px.tile([128, 8, NQ], f8, tag="x8", name="x8t")
                    nc.sync.dma_start(
                        out=st["x8"],
                        in_=x8[:, csl].rearrange("(a p) n -> p a n", p=128))
                    st["xlo"] = px.tile([128, 8, NQ], f8, tag="xlo", name="xlot")
                    nc.sync.dma_start(
                        out=st["xlo"],
                        in_=xlo[:, csl].rearrange("(a p) n -> p a n", p=128))
                    if after_xt is not None:
                        after_xt()

                def p_projqk(which, h):
                    def f():
                        if "qk" not in st:
                            st["qk"] = ps_a.tile([128, 2, NQ], f32, tag="pa",
                                                 name="qkps")
                        ps = st["qk"][:, h, :]
                        n = 0
                        for wk, xk in ((f"w{which}h", "x8"),
                                       (f"w{which}l", "x8"),
                                       (f"w{which}g", "xlo")):
                            for j in range(4):
                                nc.tensor.matmul(
                                    ps, w_sb[wk][:, j],
                                    st[xk][:, 2 * j:2 * j + 2, :],
                                    start=(n == 0), stop=(n == 11),
                                    perf_mode=DR)
                                n += 1
                    return f

                def p_rope_sin():
                    st["msin"] = ptmp.tile([128, 2, NQ], f32r, tag="msin",
                                           name="msin")
                    sn = st["tg"][:, 1, :].unsqueeze(1).to_broadcast([128, 2, NQ])
                    nc.vector.tensor_mul(st["msin"], st["qk"], sn)

                def p_rope_cos():
                    st["m1"] = ptmp.tile([128, 2, NQ], f32, tag="m1", name="m1")
                    cs = st["tg"][:, 0, :].unsqueeze(1).to_broadcast([128, 2, NQ])
                    nc.vector.tensor_mul(st["m1"], st["qk"], cs)

                def p_rope_mm():
                    st["sq"] = ps_a.tile([128, 2, NQ], f32, tag="pa",
                                         name="sqps")
                    for h in range(2):
                        nc.tensor.matmul(st["sq"][:, h, :], sperm_sb,
                                         st["msin"][:, h, :],
                                         start=True, stop=True)

                def p_rope_add():
                    st["qkstr"] = ptmp.tile([128, 2, NQ], f8, tag="qkstr",
                                            name="qkstr")
                    nc.vector.tensor_add(st["qkstr"], st["m1"], st["sq"])

                def p_qk_dma():
                    g0 = S * b + s0
                    for h in range(2):
                        for t in range(2):
                            p0 = 64 * h + 32 * t
                            nc.sync.dma_start(
                                out=qk8_sb[32 * h:32 * h + 32, t, :,
                                           g0:g0 + NQ],
                                in_=st["qkstr"][p0:p0 + 32, :, :])

                def p_projv():
                    st["v"] = ps_a.tile([128, NQ], f32, tag="pa", name="vps")
                    n = 0
                    for wk, xk in (("wvh", "x8"), ("wvl", "x8"), ("wvg", "xlo")):
                        for j in range(4):
                            nc.tensor.matmul(
                                st["v"], w_sb[wk][:, j],
                                st[xk][:, 2 * j:2 * j + 2, :],
                                start=(n == 0), stop=(n == 11), perf_mode=DR)
                            n += 1

                def p_vcopy():
                    st["vt"] = ptmp.tile([128, NQ], bf16, tag="vt", name="vt")
                    nc.scalar.copy(st["vt"], st["v"])

                def p_vdma_a():
                    nc.sync.dma_start(out=v_sb[:, 4 * u:4 * u + 4, 0:64],
                                      in_=st["vt"][0:64, :], transpose=True)

                def p_vdma_b():
                    nc.sync.dma_start(out=v_sb[:, 4 * u:4 * u + 4, 128:192],
                                      in_=st["vt"][64:128, :], transpose=True)

                return [p_load, p_projqk("q", 0), p_projqk("k", 1),
                        p_rope_sin, p_rope_cos, p_rope_mm, p_rope_add,
                        p_qk_dma, p_projv, p_vcopy, p_vdma_a, p_vdma_b]

            def proj_chunk(b, c, after_xt=None):
                for f in proj_pieces(b, c, after_xt):
                    f()

            def oproj_piece(b, c, s4):
                """Output projection for one 128-row seq tile (emitted one
                chunk late, spread across the next chunk's tiles)."""
                row0 = S * b + NQ * c + 128 * s4
                yp = ps_s.tile([128, 2, NQ], f32, tag="sps")
                for hn in range(2):
                    nc.tensor.matmul(
                        yp[:, hn, :],
                        oT_sb[:, row0:row0 + 128],
                        wo_sb[:, NQ * hn:NQ * (hn + 1)],
                        start=True, stop=True,
                    )
                ys = pys.tile([128, D_MODEL], bf16, tag="ys")
                if s4 % 2 == 0:
                    nc.vector.tensor_copy(ys, yp.rearrange("p a n -> p (a n)"))
                else:
                    nc.scalar.copy(ys, yp.rearrange("p a n -> p (a n)"))
                nc.sync.dma_start(out=y[row0:row0 + 128, :], in_=ys)

            def attn_chunk(b, c, mids=()):
                """Causal attention for query chunk c of batch b. ``mids`` are
                emitted one per attention tile (pipelined filler work)."""
                mids = list(mids)
                q0 = NQ * c
                qsl = slice(S * b + q0, S * b + q0 + NQ)
                nt = 4 * (c + 1)
                oab = ps_o.tile([128, 2, NQ], f32, tag="oacc")
                pending = []  # (p tile, j, t) awaiting PV matmul
                PV_DEPTH = 2

                def pv_flush():
                    p, j, t = pending.pop(0)
                    w0 = 128 * j
                    vt = v_sb[:, 16 * b + t, :]
                    nc.tensor.matmul(
                        oab[:, 0, w0:NQ], vt[:, 0:128], p[:, 0, w0:NQ],
                        start=(t == 0), stop=(t == nt - 1),
                    )
                    nc.tensor.matmul(
                        oab[:, 1, w0:NQ], vt[:, 128:256], p[:, 1, w0:NQ],
                        start=(t == 0), stop=(t == nt - 1),
                    )

                for t in range(nt):
                    j = max(0, t - 4 * c)  # within-chunk diagonal offset
                    w0 = 128 * j           # causally-dead query columns
                    sps = ps_s.tile([128, 2, NQ], f32, tag="sps")
                    for h in range(2):
                        base = 32 * h
                        g0 = S * b
                        nc.tensor.matmul(
                            sps[:, h, w0:NQ],
                            qk8_sb[base:base + 32, :, 1,
                                   g0 + NK * t:g0 + NK * (t + 1)],
                            qk8_sb[base:base + 32, :, 0,
                                   g0 + q0 + w0:g0 + q0 + NQ],
                            start=True, stop=True, perf_mode=DR)
                    p = pp.tile([128, 2, NQ], bf16, tag="p")
                    nc.scalar.activation(
                        p[:, :, w0:NQ], sps[:, :, w0:NQ], EXP, scale=0.125,
                    )
                    if t >= 4 * c:  # diagonal tile: mask boundary block
                        pb = p[:, :, w0:w0 + 128]
                        nc.gpsimd.tensor_mul(
                            pb, pb,
                            m128_sb.unsqueeze(1).to_broadcast([128, 2, 128]),
                        )
                    if len(pending) >= PV_DEPTH:
                        pv_flush()
                    pending.append((p, j, t))
                    if mids:
                        mids.pop(0)()
                while pending:
                    pv_flush()
                for m in mids:  # in case nt < len(mids)
                    m()

                # both heads' denominators sit replicated on partitions
                # 64:128 of oab (ones blocks in V); normalize per half
                # chunk so the trailing output projection can start early.
                rr = pr.tile([64, 2, NQ], f32, tag="rr")
                for h2 in range(2):
                    sl = slice(256 * h2, 256 * (h2 + 1))
                    qh = slice(qsl.start + 256 * h2, qsl.start + 256 * (h2 + 1))
                    nc.vector.reciprocal(rr[:, :, sl], oab[64:128, :, sl])
                    nc.vector.tensor_mul(oT_sb[0:64, qh], oab[0:64, 0, sl],
                                         rr[:, 0, sl])
                    nc.vector.tensor_mul(oT_sb[64:128, qh], oab[0:64, 1, sl],
                                         rr[:, 1, sl])

            # Software pipeline: projections run one chunk ahead of attention;
            # output projections trail their attention chunk by one.
            def oproj_mids(bc):
                if bc is None:
                    return ()
                return [lambda s4=s4: oproj_piece(bc[0], bc[1], s4)
                        for s4 in range(4)]

            prev = None  # (b, c) whose oproj is still owed
            for b in range(B):
                if b == 0:
                    proj_chunk(b, 0, after_xt=late_consts)
                for c in range(4):
                    mids = list(oproj_mids(prev))
                    if c + 1 < 4:
                        pieces = proj_pieces(b, c + 1)
                    elif b + 1 < B:
                        pieces = proj_pieces(b + 1, 0)
                    else:
                        pieces = []
                    merged = []
                    while pieces or mids:
                        if pieces:
                            merged.append(pieces.pop(0))
                        if mids:
                            merged.append(mids.pop(0))
                    attn_chunk(b, c, mids=merged)
                    prev = (b, c)
            for s4 in range(4):
                oproj_piece(prev[0], prev[1], s4)

    nc.compile()
    _RT.update(
        nc=nc, run_bass_kernel_spmd=run_bass_kernel_spmd, mybir=mybir,
    )
    return _RT


def _host_inputs(q_weight, k_weight, v_weight, o_weight, in_features):
    """Build the per-core input maps (host-side sharding + layout prep)."""
    import ml_dtypes
    f8 = ml_dtypes.float8_e4m3fn
    bf = ml_dtypes.bfloat16

    x = np.asarray(in_features, dtype=np.float32).reshape(BS, D_MODEL)
    xT = np.ascontiguousarray(x.T)
    x8 = xT.astype(f8)
    xlo = ((xT - x8.astype(np.float32)) * RS).astype(f8)

    qw = np.asarray(q_weight, dtype=np.float32)
    kw = np.asarray(k_weight, dtype=np.float32)
    vw = np.asarray(v_weight, dtype=np.float32)
    ow = np.asarray(o_weight, dtype=np.float32)

    perm64 = np.concatenate([np.arange(0, 64, 2), np.arange(1, 64, 2)])

    half = D_HEAD // 2
    inv_freq = THETA ** (-(np.arange(half, dtype=np.float64) * 2.0 / D_HEAD))
    pos = np.arange(S, dtype=np.float64)
    ang = pos[None, :] * inv_freq[:, None]        # [32, S]
    angf = np.tile(ang, (4, 1))                   # [128, S], row p -> i = p % 32
    trig = np.ascontiguousarray(
        (np.stack([np.cos(angf), np.sin(angf)], axis=1) / SC).astype(np.float32))

    spermT = np.zeros((128, 128), dtype=np.float32)
    for h in range(2):
        for i in range(32):
            spermT[h * 64 + 32 + i, h * 64 + i] = -1.0
            spermT[h * 64 + i, h * 64 + 32 + i] = 1.0

    kq = np.arange(128)
    mask128 = (np.arange(128)[None, :] >= kq[:, None]).astype(bf)

    shared = dict(x8=x8, xlo=xlo, trig=trig, sperm=spermT, mask128=mask128)

    def wsplit(A):
        # A: [1024, 128] f32, prescaled by SC
        wh = A.astype(f8)
        wl = (A - wh.astype(np.float32)).astype(f8)
        wg = (A / RS).astype(f8)
        return (np.ascontiguousarray(wh), np.ascontiguousarray(wl),
                np.ascontiguousarray(wg))

    in_maps = []
    for cidx in range(N_CORES):
        rows = slice(128 * cidx, 128 * (cidx + 1))

        def permqk(w):
            wc = w[rows]
            return np.concatenate([wc[0:64][perm64], wc[64:128][perm64]]).T * SC

        m = dict(shared)
        for nm, w in (("q", qw), ("k", kw)):
            A = permqk(w)
            for suff, arr in zip("hlg", wsplit(A)):
                m[f"w{nm}{suff}"] = arr
        Av = vw[rows].T * SC
        for suff, arr in zip("hlg", wsplit(Av)):
            m[f"wv{suff}"] = arr
        m["wo"] = np.ascontiguousarray(ow[:, rows].T / SC).astype(bf)
        in_maps.append(m)
    return in_maps


def kernel(q_weight, k_weight, v_weight, o_weight, in_features):
    rt = _build()
    in_maps = _host_inputs(q_weight, k_weight, v_weight, o_weight, in_features)
    res = rt["run_bass_kernel_spmd"](
        rt["nc"], in_maps, core_ids=list(range(N_CORES)),
    )
    y = np.zeros((BS, D_MODEL), dtype=np.float32)
    for c in range(N_CORES):
        y += np.asarray(res.results[c]["y"], dtype=np.float32)
    return y.reshape(B, S, D_MODEL)


# revision 19
# speedup vs baseline: 1.1398x; 1.0693x over previous
"""Trainium2 Bass kernel for multi-head attention (B=2, S=2048, D=1024, H=16, causal, RoPE).

Sharding: tensor-parallel over heads. Each of the 8 cores computes 2 heads
(128 of the 1024 q/k/v dims): QKV projections for its head slice, RoPE,
causal attention, and a partial output projection against its 128-column
slice of o_weight. The host sums the 8 partial outputs (the all-reduce).

v2 design notes (cost-model driven):
  - QKV projections run as fp8e4 DoubleRow matmuls (2 fp8 rows/cycle) with
    full error compensation: host splits each weight slice into
    W_hi + W_lo (both fp8, prescaled by SC=32) and streams x as
    x8 + x8lo (fp8 + scaled fp8 residual). q = x8@(Wh+Wl) + x8lo@Wh16,
    three DoubleRow chains accumulating in one PSUM tile -> 0.75x the
    bf16 matmul cost at bf16-level accuracy.
  - q and k project into one [128,2,512] PSUM tile; RoPE computes
    sin-product first (msin = psum * sin), rotates it through the sperm
    matmul (sign folded into sperm), then adds the cos-product - this
    avoids a separate PSUM->SBUF copy for the pre-rotation values.
  - Rotated q/k are written as fp8 and DMA-reshuffled (SBUF->SBUF) into
    the [32, 2-ktile, {q,k}, seq] layout that DoubleRow scores need:
    scores = K^T Q runs as one fp8 DR matmul per (tile, head) at 0.5
    cycles/column - 2x the bf16 score rate.
  - V projects in the transposed [dh, seq] orientation (same cheap DR
    shape as q/k), then a DMA-transpose instruction moves it into the
    [seq, dh] layout PV needs. V block layout [vA |ones| vB |ones] makes
    both heads' PV outputs [num; den], so the softmax denominators land
    merged on partitions 64:128 and normalize is reciprocal+2 muls.
  - exp writes P in bf16 (PV runs bf16: fp8 P fails the error budget).
  - Work is software-pipelined as in v1: projections run one chunk ahead
    of attention, output projections trail one chunk behind.
"""

import numpy as np

D_MODEL = 1024
N_HEADS = 16
D_HEAD = 64
THETA = 10000.0
B = 2
S = 2048
N_CORES = 8
BS = B * S  # 4096
NQ = 512    # query chunk width
NK = 128    # key tile width
SC = 32.0   # weight prescale into a good e4m3 binade
RS = 8.0    # x-residual prescale

_RT = {}

import os as _os, json as _json
CFG = dict(
    pv_depth=3, mask_eng="vector", norm_parts=2, ys_eng="alt2",
    lookahead2=0, merge_mode="alt", pp_bufs=6, pys_bufs=6,
    ptmp_bufs=3, px_bufs=3, pr_bufs=2, scores="fp8",
)
CFG.update(_json.loads(_os.environ.get("KCFG", "{}")))


def _build():
    if _RT:
        return _RT
    import sys
    try:
        import concourse.bass  # noqa: F401
    except ImportError:
        sys.path.insert(0, "/opt/trn_rl_repo")
    import concourse.mybir as mybir
    import concourse.tile as tile
    from concourse import bacc
    from concourse._compat import axon_active
    from concourse.bass_utils import run_bass_kernel_spmd

    f32 = mybir.dt.float32
    f32r = mybir.dt.float32r
    bf16 = mybir.dt.bfloat16
    f8 = mybir.dt.float8e4
    EXP = mybir.ActivationFunctionType.Exp
    DR = mybir.MatmulPerfMode.DoubleRow

    nc = bacc.Bacc(
        "TRN2", target_bir_lowering=False, debug=not axon_active(),
        num_devices=N_CORES,
    )

    x8 = nc.dram_tensor("x8", [D_MODEL, BS], f8, kind="ExternalInput").ap()
    xlo = nc.dram_tensor("xlo", [D_MODEL, BS], f8, kind="ExternalInput").ap()
    WKEYS = [f"w{n}{p}" for n in ("q", "k", "v") for p in ("h", "l", "g")]
    w8 = {key: nc.dram_tensor(key, [D_MODEL, 128], f8, kind="ExternalInput").ap()
          for key in WKEYS}
    wo = nc.dram_tensor("wo", [128, D_MODEL], bf16, kind="ExternalInput").ap()
    trig = nc.dram_tensor("trig", [128, 2, S], f32, kind="ExternalInput").ap()
    sperm = nc.dram_tensor("sperm", [128, 128], f32r, kind="ExternalInput").ap()
    mask128 = nc.dram_tensor("mask128", [128, 128], bf16, kind="ExternalInput").ap()
    y = nc.dram_tensor("y", [BS, D_MODEL], bf16, kind="ExternalOutput").ap()

    with tile.TileContext(nc) as tc:
        with (
            tc.tile_pool(name="singles", bufs=1) as singles,
            tc.tile_pool(name="px", bufs=CFG["px_bufs"]) as px,
            tc.tile_pool(name="ptmp", bufs=CFG["ptmp_bufs"]) as ptmp,
            tc.tile_pool(name="pp", bufs=CFG["pp_bufs"]) as pp,
            tc.tile_pool(name="pys", bufs=CFG["pys_bufs"]) as pys,
            tc.tile_pool(name="pr", bufs=CFG["pr_bufs"]) as pr,
            tc.tile_pool(name="ps_a", bufs=1, space="PSUM") as ps_a,
            tc.tile_pool(name="ps_s", bufs=2, space="PSUM") as ps_s,
            tc.tile_pool(name="ps_o", bufs=1, space="PSUM") as ps_o,
        ):
            w_sb = {key: singles.tile([128, 4, 2, 128], f8, tag=key, name=key)
                    for key in WKEYS}
            wo_sb = singles.tile([128, D_MODEL], bf16, tag="wo")
            sperm_sb = singles.tile([128, 128], f32r, tag="sperm")
            m128_sb = singles.tile([128, 128], bf16, tag="m128")
            # q/k for scores, DoubleRow layout: partition = 32*head +
            # freq, dims = [ktile(2), {q,k}, batch*S + seq]
            qk8_sb = singles.tile([64, 2, 2, BS], f8, tag="qk8")
            # bf16 fallback layout: [dh-dims(2 heads), {q,k}, batch*S+seq]
            qkb_sb = singles.tile([128, 2, BS], bf16, tag="qkb")
            # V tiles: [seq-tile partitions, 32 tiles, 256]:
            # [vA(0:64) | ones | vB(128:192) | ones]; head A lhsT = cols
            # 0:128, head B lhsT = cols 128:256 -> both PV outs [num; den].
            v_sb = singles.tile([128, 32, 256], bf16, tag="v")
            oT_sb = singles.tile([128, BS], bf16, tag="oT")

            for key in WKEYS:
                nc.scalar.dma_start(
                    out=w_sb[key],
                    in_=w8[key].rearrange("(j t p) m -> p j t m", j=4, t=2))
            nc.scalar.dma_start(out=sperm_sb, in_=sperm)
            nc.scalar.dma_start(out=m128_sb, in_=mask128)
            nc.vector.memset(v_sb[:, :, 64:128], 1.0)
            nc.vector.memset(v_sb[:, :, 192:256], 1.0)

            def late_consts():
                nc.scalar.dma_start(out=wo_sb, in_=wo)

            def proj_pieces(b, c, after_xt=None):
                """QKV projections + rope for seq chunk c of batch b (512
                positions), as a list of closures threaded through the
                attention tile loop."""
                u = 4 * b + c
                s0 = NQ * c
                csl = slice(NQ * u, NQ * (u + 1))
                st = {}

                def p_load():
                    st["tg"] = ptmp.tile([128, 2, NQ], f32, tag="tg", name="tg")
                    nc.sync.dma_start(out=st["tg"], in_=trig[:, :, s0:s0 + NQ])
                    st["x8"] = px.tile([128, 8, NQ], f8, tag="x8", name="x8t")
                    nc.sync.dma_start(
                        out=st["x8"],
                        in_=x8[:, csl].rearrange("(a p) n -> p a n", p=128))
                    st["xlo"] = px.tile([128, 8, NQ], f8, tag="xlo", name="xlot")
                    nc.sync.dma_start(
                        out=st["xlo"],
                        in_=xlo[:, csl].rearrange("(a p) n -> p a n", p=128))
                    if after_xt is not None:
                        after_xt()

                def p_projqk(which, h):
                    def f():
                        if "qk" not in st:
                            st["qk"] = ps_a.tile([128, 2, NQ], f32, tag="pa",
                                                 name="qkps")
                        ps = st["qk"][:, h, :]
                        n = 0
                        for wk, xk in ((f"w{which}h", "x8"),
                                       (f"w{which}l", "x8"),
                                       (f"w{which}g", "xlo")):
                            for j in range(4):
                                nc.tensor.matmul(
                                    ps, w_sb[wk][:, j],
                                    st[xk][:, 2 * j:2 * j + 2, :],
                                    start=(n == 0), stop=(n == 11),
                                    perf_mode=DR)
                                n += 1
                    return f

                def p_rope_sin():
                    st["msin"] = ptmp.tile([128, 2, NQ], f32r, tag="msin",
                                           name="msin")
                    sn = st["tg"][:, 1, :].unsqueeze(1).to_broadcast([128, 2, NQ])
                    nc.vector.tensor_mul(st["msin"], st["qk"], sn)

                def p_rope_cos():
                    st["m1"] = ptmp.tile([128, 2, NQ], f32, tag="m1", name="m1")
                    cs = st["tg"][:, 0, :].unsqueeze(1).to_broadcast([128, 2, NQ])
                    nc.vector.tensor_mul(st["m1"], st["qk"], cs)

                def p_rope_mm():
                    st["sq"] = ps_a.tile([128, 2, NQ], f32, tag="pa",
                                         name="sqps")
                    for h in range(2):
                        nc.tensor.matmul(st["sq"][:, h, :], sperm_sb,
                                         st["msin"][:, h, :],
                                         start=True, stop=True)

                def p_rope_add():
                    if CFG["scores"] == "fp8":
                        st["qkstr"] = ptmp.tile([128, 2, NQ], f8, tag="qkstr",
                                                name="qkstr")
                        nc.vector.tensor_add(st["qkstr"], st["m1"], st["sq"])
                    else:
                        g0 = S * b + s0
                        nc.vector.tensor_add(qkb_sb[:, :, g0:g0 + NQ],
                                             st["m1"], st["sq"])

                def p_qk_dma():
                    if CFG["scores"] != "fp8":
                        return
                    g0 = S * b + s0
                    for h in range(2):
                        for t in range(2):
                            p0 = 64 * h + 32 * t
                            nc.sync.dma_start(
                                out=qk8_sb[32 * h:32 * h + 32, t, :,
                                           g0:g0 + NQ],
                                in_=st["qkstr"][p0:p0 + 32, :, :])

                def p_projv():
                    st["v"] = ps_a.tile([128, NQ], f32, tag="pa", name="vps")
                    n = 0
                    for wk, xk in (("wvh", "x8"), ("wvl", "x8"), ("wvg", "xlo")):
                        for j in range(4):
                            nc.tensor.matmul(
                                st["v"], w_sb[wk][:, j],
                                st[xk][:, 2 * j:2 * j + 2, :],
                                start=(n == 0), stop=(n == 11), perf_mode=DR)
                            n += 1

                def p_vcopy():
                    st["vt"] = ptmp.tile([128, NQ], bf16, tag="vt", name="vt")
                    nc.scalar.copy(st["vt"], st["v"])

                def p_vdma_a():
                    nc.sync.dma_start(out=v_sb[:, 4 * u:4 * u + 4, 0:64],
                                      in_=st["vt"][0:64, :], transpose=True)

                def p_vdma_b():
                    nc.sync.dma_start(out=v_sb[:, 4 * u:4 * u + 4, 128:192],
                                      in_=st["vt"][64:128, :], transpose=True)

                return [p_load, p_projqk("q", 0), p_projqk("k", 1),
                        p_rope_sin, p_rope_cos, p_rope_mm, p_rope_add,
                        p_qk_dma, p_projv, p_vcopy, p_vdma_a, p_vdma_b]

            def proj_chunk(b, c, after_xt=None):
                for f in proj_pieces(b, c, after_xt):
                    f()

            def oproj_piece(b, c, s4):
                """Output projection for one 128-row seq tile (emitted one
                chunk late, spread across the next chunk's tiles)."""
                row0 = S * b + NQ * c + 128 * s4
                yp = ps_s.tile([128, 2, NQ], f32, tag="sps")
                for hn in range(2):
                    nc.tensor.matmul(
                        yp[:, hn, :],
                        oT_sb[:, row0:row0 + 128],
                        wo_sb[:, NQ * hn:NQ * (hn + 1)],
                        start=True, stop=True,
                    )
                ys = pys.tile([128, D_MODEL], bf16, tag="ys")
                m = CFG["ys_eng"]
                use_dve = (m == "dve" or (m == "alt" and s4 % 2 == 0)
                           or (m == "alt2" and s4 % 2 == 1))
                if use_dve:
                    nc.vector.tensor_copy(ys, yp.rearrange("p a n -> p (a n)"))
                else:
                    nc.scalar.copy(ys, yp.rearrange("p a n -> p (a n)"))
                nc.sync.dma_start(out=y[row0:row0 + 128, :], in_=ys)

            def attn_chunk(b, c, mids=()):
                """Causal attention for query chunk c of batch b. ``mids`` are
                emitted one per attention tile (pipelined filler work)."""
                mids = list(mids)
                q0 = NQ * c
                qsl = slice(S * b + q0, S * b + q0 + NQ)
                nt = 4 * (c + 1)
                oab = ps_o.tile([128, 2, NQ], f32, tag="oacc")
                pending = []  # (p tile, j, t) awaiting PV matmul
                PV_DEPTH = CFG["pv_depth"]

                def pv_flush():
                    p, j, t = pending.pop(0)
                    w0 = 128 * j
                    vt = v_sb[:, 16 * b + t, :]
                    nc.tensor.matmul(
                        oab[:, 0, w0:NQ], vt[:, 0:128], p[:, 0, w0:NQ],
                        start=(t == 0), stop=(t == nt - 1),
                    )
                    nc.tensor.matmul(
                        oab[:, 1, w0:NQ], vt[:, 128:256], p[:, 1, w0:NQ],
                        start=(t == 0), stop=(t == nt - 1),
                    )

                for t in range(nt):
                    j = max(0, t - 4 * c)  # within-chunk diagonal offset
                    w0 = 128 * j           # causally-dead query columns
                    sps = ps_s.tile([128, 2, NQ], f32, tag="sps")
                    g0 = S * b
                    for h in range(2):
                        if CFG["scores"] == "fp8":
                            base = 32 * h
                            nc.tensor.matmul(
                                sps[:, h, w0:NQ],
                                qk8_sb[base:base + 32, :, 1,
                                       g0 + NK * t:g0 + NK * (t + 1)],
                                qk8_sb[base:base + 32, :, 0,
                                       g0 + q0 + w0:g0 + q0 + NQ],
                                start=True, stop=True, perf_mode=DR)
                        else:
                            base = 64 * h
                            nc.tensor.matmul(
                                sps[:, h, w0:NQ],
                                qkb_sb[base:base + 64, 1,
                                       g0 + NK * t:g0 + NK * (t + 1)],
                                qkb_sb[base:base + 64, 0,
                                       g0 + q0 + w0:g0 + q0 + NQ],
                                start=True, stop=True)
                    p = pp.tile([128, 2, NQ], bf16, tag="p")
                    nc.scalar.activation(
                        p[:, :, w0:NQ], sps[:, :, w0:NQ], EXP, scale=0.125,
                    )
                    if t >= 4 * c:  # diagonal tile: mask boundary block
                        pb = p[:, :, w0:w0 + 128]
                        eng = nc.gpsimd if CFG["mask_eng"] == "gpsimd" else nc.vector
                        eng.tensor_mul(
                            pb, pb,
                            m128_sb.unsqueeze(1).to_broadcast([128, 2, 128]),
                        )
                    if len(pending) >= PV_DEPTH:
                        pv_flush()
                    pending.append((p, j, t))
                    if mids:
                        mids.pop(0)()
                while pending:
                    pv_flush()
                for m in mids:  # in case nt < len(mids)
                    m()

                # both heads' denominators sit replicated on partitions
                # 64:128 of oab (ones blocks in V); normalize per half
                # chunk so the trailing output projection can start early.
                rr = pr.tile([64, 2, NQ], f32, tag="rr")
                np_ = CFG["norm_parts"]
                w = NQ // np_
                for h2 in range(np_):
                    sl = slice(w * h2, w * (h2 + 1))
                    qh = slice(qsl.start + w * h2, qsl.start + w * (h2 + 1))
                    nc.vector.reciprocal(rr[:, :, sl], oab[64:128, :, sl])
                    nc.vector.tensor_mul(oT_sb[0:64, qh], oab[0:64, 0, sl],
                                         rr[:, 0, sl])
                    nc.vector.tensor_mul(oT_sb[64:128, qh], oab[0:64, 1, sl],
                                         rr[:, 1, sl])

            # Software pipeline: projections run one chunk ahead of attention;
            # output projections trail their attention chunk by one.
            def oproj_mids(bc):
                if bc is None:
                    return ()
                return [lambda s4=s4: oproj_piece(bc[0], bc[1], s4)
                        for s4 in range(4)]

            chunks = [(b, c) for b in range(B) for c in range(4)]
            la = 2 if CFG["lookahead2"] else 1
            proj_chunk(0, 0, after_xt=late_consts)
            if la == 2:
                proj_chunk(0, 1)
            for i, (b, c) in enumerate(chunks):
                pieces = (list(proj_pieces(*chunks[i + la]))
                          if i + la < len(chunks) else [])
                mids = list(oproj_mids(chunks[i - 1] if i >= 1 else None))
                mm = CFG["merge_mode"]
                if mm == "proj_first":
                    merged = pieces + mids
                elif mm == "oproj_first":
                    merged = mids + pieces
                else:
                    merged = []
                    while pieces or mids:
                        if pieces:
                            merged.append(pieces.pop(0))
                        if mids:
                            merged.append(mids.pop(0))
                attn_chunk(b, c, mids=merged)
            for s4 in range(4):
                oproj_piece(B - 1, 3, s4)

    nc.compile()
    _RT.update(
        nc=nc, run_bass_kernel_spmd=run_bass_kernel_spmd, mybir=mybir,
    )
    return _RT


def _host_inputs(q_weight, k_weight, v_weight, o_weight, in_features):
    """Build the per-core input maps (host-side sharding + layout prep)."""
    import ml_dtypes
    f8 = ml_dtypes.float8_e4m3fn
    bf = ml_dtypes.bfloat16

    x = np.asarray(in_features, dtype=np.float32).reshape(BS, D_MODEL)
    xT = np.ascontiguousarray(x.T)
    x8 = xT.astype(f8)
    xlo = ((xT - x8.astype(np.float32)) * RS).astype(f8)

    qw = np.asarray(q_weight, dtype=np.float32)
    kw = np.asarray(k_weight, dtype=np.float32)
    vw = np.asarray(v_weight, dtype=np.float32)
    ow = np.asarray(o_weight, dtype=np.float32)

    perm64 = np.concatenate([np.arange(0, 64, 2), np.arange(1, 64, 2)])

    half = D_HEAD // 2
    inv_freq = THETA ** (-(np.arange(half, dtype=np.float64) * 2.0 / D_HEAD))
    pos = np.arange(S, dtype=np.float64)
    ang = pos[None, :] * inv_freq[:, None]        # [32, S]
    angf = np.tile(ang, (4, 1))                   # [128, S], row p -> i = p % 32
    trig = np.ascontiguousarray(
        (np.stack([np.cos(angf), np.sin(angf)], axis=1) / SC).astype(np.float32))

    spermT = np.zeros((128, 128), dtype=np.float32)
    for h in range(2):
        for i in range(32):
            spermT[h * 64 + 32 + i, h * 64 + i] = -1.0
            spermT[h * 64 + i, h * 64 + 32 + i] = 1.0

    kq = np.arange(128)
    mask128 = (np.arange(128)[None, :] >= kq[:, None]).astype(bf)

    shared = dict(x8=x8, xlo=xlo, trig=trig, sperm=spermT, mask128=mask128)

    def wsplit(A):
        # A: [1024, 128] f32, prescaled by SC
        wh = A.astype(f8)
        wl = (A - wh.astype(np.float32)).astype(f8)
        wg = (A / RS).astype(f8)
        return (np.ascontiguousarray(wh), np.ascontiguousarray(wl),
                np.ascontiguousarray(wg))

    in_maps = []
    for cidx in range(N_CORES):
        rows = slice(128 * cidx, 128 * (cidx + 1))

        def permqk(w):
            wc = w[rows]
            return np.concatenate([wc[0:64][perm64], wc[64:128][perm64]]).T * SC

        m = dict(shared)
        for nm, w in (("q", qw), ("k", kw)):
            A = permqk(w)
            for suff, arr in zip("hlg", wsplit(A)):
                m[f"w{nm}{suff}"] = arr
        Av = vw[rows].T * SC
        for suff, arr in zip("hlg", wsplit(Av)):
            m[f"wv{suff}"] = arr
        m["wo"] = np.ascontiguousarray(ow[:, rows].T / SC).astype(bf)
        in_maps.append(m)
    return in_maps


def kernel(q_weight, k_weight, v_weight, o_weight, in_features):
    rt = _build()
    in_maps = _host_inputs(q_weight, k_weight, v_weight, o_weight, in_features)
    res = rt["run_bass_kernel_spmd"](
        rt["nc"], in_maps, core_ids=list(range(N_CORES)),
    )
    y = np.zeros((BS, D_MODEL), dtype=np.float32)
    for c in range(N_CORES):
        y += np.asarray(res.results[c]["y"], dtype=np.float32)
    return y.reshape(B, S, D_MODEL)
